# revision 1
# baseline (speedup 1.0000x reference)
"""Causal self-attention (B=2, L=4096, D=768, H=12) on 8 TRN2 NeuronCores.

Sharding: core c -> batch b = c//4, head group g = c%4 (heads 3g..3g+2).

Per-core structure:
- Upfront QKV projection: q/k weight-stationary (wqkv chunks stationary,
  x moving, transposed qT/kT layout lands directly); v x-tile-stationary
  (natural [token, dh] layout); small weights DMA'd before the big x
  tensor so the first matmul starts early.
- Flash-style causal attention with scores^T layout, two streams per
  pair staggered as alternating half-chunks: while exp of half-chunk h
  runs on the scalar engine, the PE computes attn@v of h-2 (same
  stream) and scores of h+1 (other stream, other PSUM banks), so the
  scalar engine stays saturated and the six score banks double-buffer
  across streams.  Diagonal tiles are trimmed to their live columns
  (bank-aligned segments); a single [128,128] triangle mask handles the
  boundary.  Rowsums ride along as a fused ones-column (M=65 attn@v).
- Output projection with h0/h1 stacked in the partition dim (K=128
  matmul) plus a K=64 h2 accumulate, woven between the next phase's
  attention pairs; 5 ReduceScatter chunks (last one half-size) with the
  final out-DMAs on the gpsimd queue so they never head-of-line block
  the sync DMA queue.
Host reassembles the full [2, 4096, 768] output and adds bo (zeros per
problem spec; kernel() fails loudly if not).
"""

import sys

for _p in ("/opt/trn_rl_repo",):
    if _p not in sys.path:
        sys.path.insert(0, _p)

import numpy as np
import ml_dtypes

B, L, D, H = 2, 4096, 768, 12
Dh = D // H          # 64
HPC = 3              # heads per core
NCORES = 8
QB = 512             # q block
KT = 128             # k tile
NQ = L // QB         # 8
NKT = L // KT        # 32
KC = D // 128        # 6 contraction chunks for projections
CH = 3               # k-tiles per exp chunk (3 PSUM banks)

_CACHE = {}


def _tiles_for_block(i):
    """(kb, width, qoff) per k-tile for q-block i, diag tiles trimmed."""
    out = []
    for kb in range(4 * (i + 1)):
        r = kb - 4 * i
        w = QB if r < 0 else QB - KT * r
        out.append((kb, w, QB - w))
    return out


def _chunks_for_block(i):
    """Chunks of <=CH tiles, in forward order.

    Forward order keeps the first attn@v matmul (start=True) full-width:
    tile kb=0 always has w=QB, so the whole py bank range is written
    before any trimmed accumulate touches a sub-range."""
    tiles = _tiles_for_block(i)
    return [tiles[c0:c0 + CH] for c0 in range(0, len(tiles), CH)]


def _build():
    import concourse.mybir as mybir
    import concourse.tile as tile
    from concourse import bacc

    bf16 = mybir.dt.bfloat16
    f32 = mybir.dt.float32
    Exp = mybir.ActivationFunctionType.Exp

    nc = bacc.Bacc("TRN2", target_bir_lowering=False, debug=False,
                   num_devices=NCORES)

    xT = nc.dram_tensor('xT', [D, L], bf16, kind='ExternalInput')
    wqkv = nc.dram_tensor('wqkv', [D, 576], bf16, kind='ExternalInput')
    wo01 = nc.dram_tensor('wo01', [128, D], bf16, kind='ExternalInput')
    wo2 = nc.dram_tensor('wo2', [64, D], bf16, kind='ExternalInput')
    msk = nc.dram_tensor('msk', [KT, KT], bf16, kind='ExternalInput')
    out = nc.dram_tensor('out', [4 * 256, D], bf16, kind='ExternalOutput')

    with tile.TileContext(nc) as tc:
        with tc.tile_pool(name='const', bufs=1) as cpool, \
             tc.tile_pool(name='work', bufs=3) as wpool, \
             tc.tile_pool(name='dram', bufs=1, space='DRAM') as dp:

            # ---------------- load phase ----------------
            # small weight tensors first so the first projection matmul
            # isn't queued behind 6 MB of x chunks
            wq_sb = cpool.tile([128, KC, 576], bf16)
            for kc in range(KC):
                nc.sync.dma_start(out=wq_sb[:, kc, :], in_=wqkv[kc * 128:(kc + 1) * 128, :])
            wo01_sb = cpool.tile([128, D], bf16)
            nc.sync.dma_start(out=wo01_sb[:, :], in_=wo01[:, :])
            wo2_sb = cpool.tile([64, D], bf16)
            nc.sync.dma_start(out=wo2_sb[:, :], in_=wo2[:, :])
            msk_sb = cpool.tile([KT, KT], bf16)
            nc.sync.dma_start(out=msk_sb[:, :], in_=msk[:, :])
            xt = cpool.tile([128, KC, L], bf16)
            for kc in range(KC):
                nc.sync.dma_start(out=xt[:, kc, :], in_=xT[kc * 128:(kc + 1) * 128, :])
            ones = cpool.tile([128, 64], bf16)
            nc.vector.memset(ones[:, :], 1.0)

            # qkA: p0-63 = [q_h0 | k_h0], p64-127 = [q_h1 | k_h1]
            # qk2: p0-63 = [q_h2 | k_h2], p64-127 = duplicate
            qkA = cpool.tile([128, 2 * L], bf16)
            qk2 = cpool.tile([128, 2 * L], bf16)
            v_sb = cpool.tile([128, NKT, HPC, 65], bf16)
            nc.vector.memset(v_sb[:, :, :, 64:65], 1.0)
            yt01 = cpool.tile([128, L], bf16)
            yt2 = cpool.tile([64, L], bf16)
            yt = [yt01[0:64, :], yt01[64:128, :], yt2[0:64, :]]

            # ------- QKV projection pieces -------
            # q/k: weight-stationary (wqkv chunk stationary, x moving).
            # v: x-tile stationary, wv moving -> natural [token, dh] layout.
            def consume_qk(ct, n, ps):
                tsl = slice(n * QB, (n + 1) * QB)
                if ct == 0:
                    nc.vector.tensor_copy(qkA[:, tsl], ps[:, 0:QB])
                elif ct == 1:
                    nc.vector.tensor_copy(qkA[:, L + n * QB:L + (n + 1) * QB],
                                          ps[:, 0:QB])
                else:
                    st = wpool.tile([128, QB], bf16, tag='st', name='st')
                    nc.vector.tensor_copy(st[:, :], ps[:, 0:QB])
                    # same-partition copies on DVE; cross-partition
                    # duplicates via DMA (DVE cannot cross lanes)
                    nc.vector.tensor_copy(qk2[0:64, tsl], st[0:64, :])
                    nc.sync.dma_start(out=qk2[64:128, tsl], in_=st[0:64, :])
                    nc.sync.dma_start(
                        out=qk2[0:64, L + n * QB:L + (n + 1) * QB],
                        in_=st[64:128, :])
                    nc.vector.tensor_copy(
                        qk2[64:128, L + n * QB:L + (n + 1) * QB],
                        st[64:128, :])

            def emit_v(pool, m, tag):
                psv = pool.tile([128, QB], f32, tag=tag, bufs=1, name='psv')
                for kc in range(KC):
                    nc.tensor.matmul(psv[:, 0:192],
                                     xt[:, kc, m * 128:(m + 1) * 128],
                                     wq_sb[:, kc, 384:576],
                                     start=(kc == 0), stop=(kc == KC - 1))
                nc.vector.tensor_copy(v_sb[:, m, :, 0:64], psv[:, 0:192])

            def proj_head(pjp):
                """Everything attention phase 0 needs: q/k/qk2 for blocks
                0-1 and v tiles 0-7."""
                for ct in range(3):
                    pss = [pjp.tile([128, QB], f32, tag=f'pj{n}', bufs=1,
                                    name=f'pj{n}') for n in (0, 1)]
                    for kc in range(KC):
                        for n in (0, 1):
                            nc.tensor.matmul(
                                pss[n][:, 0:QB],
                                wq_sb[:, kc, ct * 128:ct * 128 + 128],
                                xt[:, kc, n * QB:(n + 1) * QB],
                                start=(kc == 0), stop=(kc == KC - 1))
                    for n in (0, 1):
                        consume_qk(ct, n, pss[n])
                for m in range(8):
                    emit_v(pjp, m, f'pj{2 + m % 6}')

            def proj_pair(na, nb):
                """q/k/qk2 for blocks na, nb using the attention pool's
                spare py banks (interleaved into attention phase 0)."""
                for ct in range(3):
                    pss = [pp.tile([128, QB], f32, tag=t, bufs=1, name='pjt')
                           for t in ('yA', 'yB')]
                    for kc in range(KC):
                        for j, n in enumerate((na, nb)):
                            nc.tensor.matmul(
                                pss[j][:, 0:QB],
                                wq_sb[:, kc, ct * 128:ct * 128 + 128],
                                xt[:, kc, n * QB:(n + 1) * QB],
                                start=(kc == 0), stop=(kc == KC - 1))
                    for j, n in enumerate((na, nb)):
                        consume_qk(ct, n, pss[j])

            def proj_v_range(m0, m1):
                for m in range(m0, m1):
                    emit_v(pp, m, ('yA', 'yB')[m % 2])

            # ---------------- attention ----------------
            def normalize(py, yt_t, qsl, stag):
                rs = wpool.tile([1, QB], bf16, tag='rs', name='rs')
                nc.vector.tensor_copy(rs[:, :], py[64:65, :])
                pb = pp.tile([128, CH * QB], f32, tag=stag, bufs=1, name='pb')
                nc.tensor.matmul(pb[0:64, 0:QB], ones[0:1, 0:64], rs[0:1, :],
                                 start=True, stop=True)
                rcp = wpool.tile([64, QB], f32, tag='rcp', name='rcp')
                nc.vector.reciprocal_approx_fast(out=rcp[:, :], in_=pb[0:64, 0:QB])
                nc.vector.tensor_mul(yt_t[:, qsl], py[0:64, :], rcp[:, :])

            def attn_pair(qA, kA, iA, ytA, hA, qB, kB, iB, ytB, hB):
                """Two causal streams, staggered half-chunk pipeline.

                Half-chunks alternate A(0) B(0) A(1) B(1)...; while exp of
                half-chunk h runs on ACT, the PE computes attn@v of h-2
                (same stream, pt ready) and scores of h+1 (other stream,
                other PSUM banks) - so ACT never starves and the 6 score
                banks double-buffer across streams.
                """
                pyA = pp.tile([128, QB], f32, tag='yA', bufs=1, name='pyA')
                pyB = pp.tile([128, QB], f32, tag='yB', bufs=1, name='pyB')
                chA = _chunks_for_block(iA)
                chB = _chunks_for_block(iB)
                nch = max(len(chA), len(chB))
                chA = [[]] * (nch - len(chA)) + chA
                chB = [[]] * (nch - len(chB)) + chB
                seq = []
                for j in range(nch):
                    seq.append(('A', chA[j]))
                    seq.append(('B', chB[j]))
                ctx = {
                    'A': (pyA, qA, kA, iA, hA, ytA, 'sA'),
                    'B': (pyB, qB, kB, iB, hB, ytB, 'sB'),
                }
                first = {'A': True, 'B': True}
                last_idx = {'A': None, 'B': None}
                for h, (X, tiles) in enumerate(seq):
                    if tiles:
                        last_idx[X] = h

                def emit_av(X, tiles, pt, is_last):
                    pyX, _, _, _, hX, _, _ = ctx[X]
                    for t, (kb, w, qo) in enumerate(tiles):
                        nc.tensor.matmul(
                            pyX[0:65, qo:qo + w],
                            v_sb[:, kb, hX, 0:65],
                            pt[:, t * QB + qo:t * QB + qo + w],
                            start=first[X] and t == 0,
                            stop=is_last and t == len(tiles) - 1)
                        if t == 0:
                            first[X] = False

                pend = {}  # h -> (X, tiles, pt)
                for h, (X, tiles) in enumerate(seq):
                    pyX, qX, kX, iX, hX, ytX, stag = ctx[X]
                    if tiles:
                        # segment t lives at cols [t*QB + qo, (t+1)*QB) of
                        # its chunk tile - bank-aligned, trimmed to the live
                        # causal range.  Dead columns are exp'd but never
                        # read by attn@v.
                        s = pp.tile([128, CH * QB], f32, tag=stag, bufs=1,
                                    name='s' + X)
                        for t, (kb, w, qo) in enumerate(tiles):
                            nc.tensor.matmul(
                                s[:, t * QB + qo:t * QB + qo + w],
                                kX[:, kb * KT:(kb + 1) * KT],
                                qX[:, iX * QB + qo:iX * QB + qo + w],
                                start=True, stop=True)
                    # attn@v of h-2 (same stream) keeps PE busy during exp(h)
                    if h - 2 in pend:
                        Xp, tp, ptp = pend.pop(h - 2)
                        emit_av(Xp, tp, ptp, last_idx[Xp] == h - 2)
                    if tiles:
                        wX = len(tiles) * QB
                        pt = wpool.tile([128, CH * QB], bf16, tag='pt', bufs=8,
                                        name='pt' + X)
                        nc.scalar.activation(pt[:, 0:wX], s[:, 0:wX], Exp)
                        for t, (kb, w, qo) in enumerate(tiles):
                            if kb - 4 * iX >= 0:
                                nc.vector.tensor_mul(
                                    pt[:, t * QB + qo:t * QB + qo + KT],
                                    pt[:, t * QB + qo:t * QB + qo + KT],
                                    msk_sb[:, :])
                        pend[h] = (X, tiles, pt)
                # drain
                for h in sorted(pend):
                    Xp, tp, ptp = pend[h]
                    emit_av(Xp, tp, ptp, last_idx[Xp] == h)
                normalize(pyA, ytA, slice(iA * QB, (iA + 1) * QB), 'sA')
                normalize(pyB, ytB, slice(iB * QB, (iB + 1) * QB), 'sB')

            # ---------- main loop ----------
            groups = [[0, 1, 2, 3], [4, 5, 6, 7]]
            import concourse.mybir as _mybir
            # output row-chunks: blocks 0-1, 2-3, 4-5 then 6 and 7 split so
            # the final collective is half-size (shorter serial tail)
            pchs = [dp.tile([2 * QB, D], bf16, name='pch0'),
                    dp.tile([2 * QB, D], bf16, name='pch1'),
                    dp.tile([2 * QB, D], bf16, name='pch2'),
                    dp.tile([QB, D], bf16, name='pch3a'),
                    dp.tile([QB, D], bf16, name='pch3b')]
            rschs = [dp.tile([256, D], bf16, name='rsch0'),
                     dp.tile([256, D], bf16, name='rsch1'),
                     dp.tile([256, D], bf16, name='rsch2'),
                     dp.tile([128, D], bf16, name='rsch3a'),
                     dp.tile([128, D], bf16, name='rsch3b')]
            out_rows = [(0, 256), (256, 256), (512, 256), (768, 128),
                        (896, 128)]

            def outproj_block(i, ci, ib):
                for mm in range(QB // 128):
                    tok = i * QB + mm * 128
                    ot = wpool.tile([128, D], bf16, tag='ot', name='ot')
                    pos = [pp.tile([128, QB], f32, tag=('yA', 'yB')[dj],
                                   bufs=1, name='po') for dj in range(2)]
                    for dj, (d0, dw) in enumerate(((0, QB), (QB, 256))):
                        nc.tensor.matmul(pos[dj][:, 0:dw],
                                         yt01[:, tok:tok + 128],
                                         wo01_sb[:, d0:d0 + dw],
                                         start=True, stop=False)
                    for dj, (d0, dw) in enumerate(((0, QB), (QB, 256))):
                        nc.tensor.matmul(pos[dj][:, 0:dw],
                                         yt2[:, tok:tok + 128],
                                         wo2_sb[:, d0:d0 + dw],
                                         start=False, stop=True)
                    for dj, (d0, dw) in enumerate(((0, QB), (QB, 256))):
                        nc.vector.tensor_copy(ot[:, d0:d0 + dw],
                                              pos[dj][:, 0:dw])
                    row = (i - ib) * QB + mm * 128
                    nc.sync.dma_start(out=pchs[ci][row:row + 128, :],
                                      in_=ot[:, :])

            def reduce_scatter(ci):
                nc.gpsimd.collective_compute(
                    "ReduceScatter", _mybir.AluOpType.add,
                    replica_groups=groups, ins=[pchs[ci].opt()],
                    outs=[rschs[ci].opt()])
                # gpsimd queue: serializes after its own RS without
                # head-of-line blocking the sync DMA queue
                r0, rn = out_rows[ci]
                nc.gpsimd.dma_start(out=out[r0:r0 + rn, :], in_=rschs[ci][:, :])

            def attn_h01(i):
                attn_pair(qkA[0:64, 0:L], qkA[0:64, L:2 * L], i, yt[0], 0,
                          qkA[64:128, 0:L], qkA[64:128, L:2 * L], i, yt[1], 1)

            def attn_h2(p):
                attn_pair(qk2[0:64, 0:L], qk2[0:64, L:2 * L], 2 * p, yt[2], 2,
                          qk2[64:128, 0:L], qk2[64:128, L:2 * L], 2 * p + 1,
                          yt[2], 2)

            with tc.tile_pool(name='psum_proj', bufs=1, space='PSUM') as pjp:
                proj_head(pjp)

            with tc.tile_pool(name='psum', bufs=1, space='PSUM') as pp_:
                pp = pp_
                # phase 0 with the projection tail interleaved: the PE does
                # proj matmuls while ACT chews phase-0 exps
                attn_h01(0)
                proj_pair(2, 3)
                proj_v_range(8, 16)
                attn_h01(1)
                proj_pair(4, 5)
                proj_v_range(16, 24)
                attn_h2(0)
                proj_pair(6, 7)
                proj_v_range(24, 32)
                for p in (1, 2):
                    attn_h01(2 * p)
                    outproj_block(2 * (p - 1), p - 1, 2 * (p - 1))
                    attn_h01(2 * p + 1)
                    outproj_block(2 * p - 1, p - 1, 2 * (p - 1))
                    reduce_scatter(p - 1)
                    attn_h2(p)
                # phase 3 reordered: h2 before h01(7) so block 6 completes
                # early and its outproj+RS overlap the last attention pair
                attn_h01(6)
                outproj_block(4, 2, 4)
                attn_h2(3)
                outproj_block(5, 2, 4)
                reduce_scatter(2)
                outproj_block(6, 3, 6)
                reduce_scatter(3)
                attn_h01(7)
                outproj_block(7, 4, 7)
                reduce_scatter(4)
    nc.compile()
    return nc


def kernel(x, Wqkv, bqkv, Wo, bo):
    from concourse.bass_utils import run_bass_kernel_spmd

    if 'nc' not in _CACHE:
        _CACHE['nc'] = _build()
    nc = _CACHE['nc']

    bf = ml_dtypes.bfloat16
    x = np.asarray(x, np.float32)
    Wqkv = np.asarray(Wqkv, np.float32)
    bqkv = np.asarray(bqkv, np.float32)
    Wo = np.asarray(Wo, np.float32)
    bo = np.asarray(bo, np.float32)

    # device graph omits the qkv bias adds (always zeros per problem spec)
    assert np.abs(bqkv).max() == 0.0, "nonzero bqkv unsupported by this kernel"

    scale = 1.0 / np.sqrt(Dh)
    Q, K, V = Wqkv[:, 0:D], Wqkv[:, D:2 * D], Wqkv[:, 2 * D:3 * D]

    kl = np.arange(KT)[:, None]
    ql = np.arange(KT)[None, :]
    msk = np.ascontiguousarray(ql >= kl).astype(bf)

    in_maps = []
    for c in range(NCORES):
        b, g = divmod(c, 4)
        hs = [3 * g, 3 * g + 1, 3 * g + 2]
        cols = lambda W, h: W[:, h * Dh:(h + 1) * Dh]
        wqkv_np = np.concatenate(
            [cols(Q, hs[0]) * scale, cols(Q, hs[1]) * scale,
             cols(K, hs[0]), cols(K, hs[1]),
             cols(Q, hs[2]) * scale, cols(K, hs[2]),
             cols(V, hs[0]), cols(V, hs[1]), cols(V, hs[2])],
            axis=1).astype(bf)
        wo01_np = Wo[3 * g * Dh:(3 * g + 2) * Dh, :].astype(bf)
        wo2_np = Wo[(3 * g + 2) * Dh:(3 * g + 3) * Dh, :].astype(bf)
        xT_np = np.ascontiguousarray(x[b].T).astype(bf)
        in_maps.append({
            'xT': xT_np, 'wqkv': np.ascontiguousarray(wqkv_np),
            'wo01': np.ascontiguousarray(wo01_np),
            'wo2': np.ascontiguousarray(wo2_np),
            'msk': msk,
        })

    res = run_bass_kernel_spmd(nc, in_maps, core_ids=list(range(NCORES)))

    out = np.empty((B, L, D), np.float32)
    # (device out-row start, global q start, per-core rows)
    chunk_map = [(0, 0, 256), (256, 1024, 256), (512, 2048, 256),
                 (768, 3072, 128), (896, 3584, 128)]
    for c in range(NCORES):
        b, g = divmod(c, 4)
        o = res.results[c]['out'].astype(np.float32)
        for r0, q0, rn in chunk_map:
            out[b, q0 + g * rn:q0 + (g + 1) * rn, :] = o[r0:r0 + rn, :]
    out += bo[None, None, :]
    return out



# revision 5
# speedup vs baseline: 1.1332x; 1.1332x over previous
"""Causal self-attention (B=2, L=4096, D=768, H=12) on 8 TRN2 NeuronCores.

Sharding: core c -> batch b = c//4, head group g = c%4 (heads 3g..3g+2).

Per-core structure:
- Upfront QKV projection: q/k weight-stationary (wqkv chunks stationary,
  x moving, transposed qT/kT layout lands directly); v x-tile-stationary
  (natural [token, dh] layout); small weights DMA'd before the big x
  tensor so the first matmul starts early.
- Flash-style causal attention with scores^T layout, two streams per
  pair staggered as alternating half-chunks: while exp of half-chunk h
  runs on the scalar engine, the PE computes attn@v of h-2 (same
  stream) and scores of h+1 (other stream, other PSUM banks), so the
  scalar engine stays saturated and the six score banks double-buffer
  across streams.  Diagonal tiles are trimmed to their live columns
  (bank-aligned segments); a single [128,128] triangle mask handles the
  boundary.  Rowsums ride along as a fused ones-column (M=65 attn@v).
- Output projection with h0/h1 stacked in the partition dim (K=128
  matmul) plus a K=64 h2 accumulate, woven between the next phase's
  attention pairs; 5 ReduceScatter chunks (last one half-size) with the
  final out-DMAs on the gpsimd queue so they never head-of-line block
  the sync DMA queue.
Host reassembles the full [2, 4096, 768] output and adds bo (zeros per
problem spec; kernel() fails loudly if not).
"""

import sys

for _p in ("/opt/trn_rl_repo",):
    if _p not in sys.path:
        sys.path.insert(0, _p)

import numpy as np
import ml_dtypes

B, L, D, H = 2, 4096, 768, 12
Dh = D // H          # 64
HPC = 3              # heads per core
NCORES = 8
QB = 512             # q block
KT = 128             # k tile
NQ = L // QB         # 8
NKT = L // KT        # 32
KC = D // 128        # 6 contraction chunks for projections
CH = 3               # k-tiles per exp chunk (3 PSUM banks)

_CACHE = {}


def _tiles_for_block(i):
    """(kb, width, qoff) per k-tile for q-block i, diag tiles trimmed."""
    out = []
    for kb in range(4 * (i + 1)):
        r = kb - 4 * i
        w = QB if r < 0 else QB - KT * r
        out.append((kb, w, QB - w))
    return out


def _chunks_for_block(i):
    """Chunks of <=CH tiles, in forward order.

    Forward order keeps the first attn@v matmul (start=True) full-width:
    tile kb=0 always has w=QB, so the whole py bank range is written
    before any trimmed accumulate touches a sub-range."""
    tiles = _tiles_for_block(i)
    return [tiles[c0:c0 + CH] for c0 in range(0, len(tiles), CH)]


def _build():
    import concourse.mybir as mybir
    import concourse.tile as tile
    from concourse import bacc

    bf16 = mybir.dt.bfloat16
    f32 = mybir.dt.float32
    Exp = mybir.ActivationFunctionType.Exp

    nc = bacc.Bacc("TRN2", target_bir_lowering=False, debug=False,
                   num_devices=NCORES)

    xT = nc.dram_tensor('xT', [D, L], bf16, kind='ExternalInput')
    wqkv = nc.dram_tensor('wqkv', [D, 576], bf16, kind='ExternalInput')
    wo01 = nc.dram_tensor('wo01', [128, D], bf16, kind='ExternalInput')
    wo2 = nc.dram_tensor('wo2', [64, D], bf16, kind='ExternalInput')
    msk = nc.dram_tensor('msk', [KT, KT], bf16, kind='ExternalInput')
    # full-length per-core PARTIAL outproj sums; host adds the 4 cores of a
    # batch group (replaces the ReduceScatter collectives entirely)
    out = nc.dram_tensor('out', [L, D], bf16, kind='ExternalOutput')

    with tile.TileContext(nc) as tc:
        with tc.tile_pool(name='const', bufs=1) as cpool, \
             tc.tile_pool(name='work', bufs=3) as wpool, \
             tc.tile_pool(name='dram', bufs=1, space='DRAM') as dp:

            # ---------------- load phase ----------------
            # small weight tensors first so the first projection matmul
            # isn't queued behind 6 MB of x chunks
            wq_sb = cpool.tile([128, KC, 576], bf16)
            for kc in range(KC):
                nc.sync.dma_start(out=wq_sb[:, kc, :], in_=wqkv[kc * 128:(kc + 1) * 128, :])
            msk_sb = cpool.tile([KT, KT], bf16)
            nc.sync.dma_start(out=msk_sb[:, :], in_=msk[:, :])
            # x arrives in 1024-col pieces so the first projection matmuls
            # (which need only tokens 0:1024) start ~4us in, not ~20us
            xt = cpool.tile([128, KC, L], bf16)
            for p0 in range(0, L, 1024):
                for kc in range(KC):
                    nc.sync.dma_start(out=xt[:, kc, p0:p0 + 1024],
                                      in_=xT[kc * 128:(kc + 1) * 128, p0:p0 + 1024])
            wo01_sb = cpool.tile([128, D], bf16)
            nc.sync.dma_start(out=wo01_sb[:, :], in_=wo01[:, :])
            wo2_sb = cpool.tile([64, D], bf16)
            nc.sync.dma_start(out=wo2_sb[:, :], in_=wo2[:, :])
            ones = cpool.tile([128, 64], bf16)
            nc.vector.memset(ones[:, :], 1.0)

            # qkA: p0-63 = [q_h0 | k_h0], p64-127 = [q_h1 | k_h1]
            # qk2: p0-63 = [q_h2 | k_h2], p64-127 = duplicate
            qkA = cpool.tile([128, 2 * L], bf16)
            qk2 = cpool.tile([128, 2 * L], bf16)
            v_sb = cpool.tile([128, NKT, HPC, 65], bf16)
            nc.vector.memset(v_sb[:, :, :, 64:65], 1.0)
            yt01 = cpool.tile([128, L], bf16)
            yt2 = cpool.tile([64, L], bf16)
            yt = [yt01[0:64, :], yt01[64:128, :], yt2[0:64, :]]

            # ------- QKV projection pieces -------
            # q/k: weight-stationary (wqkv chunk stationary, x moving).
            # v: x-tile stationary, wv moving -> natural [token, dh] layout.
            def consume_qk(ct, n, ps):
                tsl = slice(n * QB, (n + 1) * QB)
                if ct == 0:
                    nc.vector.tensor_copy(qkA[:, tsl], ps[:, 0:QB])
                elif ct == 1:
                    nc.vector.tensor_copy(qkA[:, L + n * QB:L + (n + 1) * QB],
                                          ps[:, 0:QB])
                else:
                    st = wpool.tile([128, QB], bf16, tag='st', name='st')
                    nc.vector.tensor_copy(st[:, :], ps[:, 0:QB])
                    # same-partition copies on DVE; cross-partition
                    # duplicates via DMA (DVE cannot cross lanes)
                    nc.vector.tensor_copy(qk2[0:64, tsl], st[0:64, :])
                    nc.sync.dma_start(out=qk2[64:128, tsl], in_=st[0:64, :])
                    nc.sync.dma_start(
                        out=qk2[0:64, L + n * QB:L + (n + 1) * QB],
                        in_=st[64:128, :])
                    nc.vector.tensor_copy(
                        qk2[64:128, L + n * QB:L + (n + 1) * QB],
                        st[64:128, :])

            def emit_v(pool, m, tag):
                psv = pool.tile([128, QB], f32, tag=tag, bufs=1, name='psv')
                for kc in range(KC):
                    nc.tensor.matmul(psv[:, 0:192],
                                     xt[:, kc, m * 128:(m + 1) * 128],
                                     wq_sb[:, kc, 384:576],
                                     start=(kc == 0), stop=(kc == KC - 1))
                nc.vector.tensor_copy(v_sb[:, m, :, 0:64], psv[:, 0:192])

            def proj_head(pjp):
                """Everything attention phase 0 needs: q/k/qk2 for blocks
                0-1 and v tiles 0-7."""
                for ct in range(3):
                    pss = [pjp.tile([128, QB], f32, tag=f'pj{n}', bufs=1,
                                    name=f'pj{n}') for n in (0, 1)]
                    for kc in range(KC):
                        for n in (0, 1):
                            nc.tensor.matmul(
                                pss[n][:, 0:QB],
                                wq_sb[:, kc, ct * 128:ct * 128 + 128],
                                xt[:, kc, n * QB:(n + 1) * QB],
                                start=(kc == 0), stop=(kc == KC - 1))
                    for n in (0, 1):
                        consume_qk(ct, n, pss[n])
                for m in range(8):
                    emit_v(pjp, m, f'pj{2 + m % 6}')

            def proj_pair(na, nb):
                """q/k/qk2 for blocks na, nb using the attention pool's
                spare py banks (interleaved into attention phase 0)."""
                for ct in range(3):
                    pss = [pp.tile([128, QB], f32, tag=t, bufs=1, name='pjt')
                           for t in ('yA', 'yB')]
                    for kc in range(KC):
                        for j, n in enumerate((na, nb)):
                            nc.tensor.matmul(
                                pss[j][:, 0:QB],
                                wq_sb[:, kc, ct * 128:ct * 128 + 128],
                                xt[:, kc, n * QB:(n + 1) * QB],
                                start=(kc == 0), stop=(kc == KC - 1))
                    for j, n in enumerate((na, nb)):
                        consume_qk(ct, n, pss[j])

            def proj_v_range(m0, m1):
                for m in range(m0, m1):
                    emit_v(pp, m, ('yA', 'yB')[m % 2])

            # ---------------- attention ----------------
            def normalize(py, yt_t, qsl, stag):
                rs = wpool.tile([1, QB], bf16, tag='rs', name='rs')
                nc.vector.tensor_copy(rs[:, :], py[64:65, :])
                pb = pp.tile([128, CH * QB], f32, tag=stag, bufs=1, name='pb')
                nc.tensor.matmul(pb[0:64, 0:QB], ones[0:1, 0:64], rs[0:1, :],
                                 start=True, stop=True)
                rcp = wpool.tile([64, QB], f32, tag='rcp', name='rcp')
                nc.vector.reciprocal_approx_fast(out=rcp[:, :], in_=pb[0:64, 0:QB])
                nc.vector.tensor_mul(yt_t[:, qsl], py[0:64, :], rcp[:, :])

            def attn_pair(qA, kA, iA, ytA, hA, qB, kB, iB, ytB, hB):
                """Two causal streams, staggered half-chunk pipeline.

                Half-chunks alternate A(0) B(0) A(1) B(1)...; while exp of
                half-chunk h runs on ACT, the PE computes attn@v of h-2
                (same stream, pt ready) and scores of h+1 (other stream,
                other PSUM banks) - so ACT never starves and the 6 score
                banks double-buffer across streams.
                """
                pyA = pp.tile([128, QB], f32, tag='yA', bufs=1, name='pyA')
                pyB = pp.tile([128, QB], f32, tag='yB', bufs=1, name='pyB')
                chA = _chunks_for_block(iA)
                chB = _chunks_for_block(iB)
                nch = max(len(chA), len(chB))
                chA = [[]] * (nch - len(chA)) + chA
                chB = [[]] * (nch - len(chB)) + chB
                seq = []
                for j in range(nch):
                    seq.append(('A', chA[j]))
                    seq.append(('B', chB[j]))
                ctx = {
                    'A': (pyA, qA, kA, iA, hA, ytA, 'sA'),
                    'B': (pyB, qB, kB, iB, hB, ytB, 'sB'),
                }
                first = {'A': True, 'B': True}
                last_idx = {'A': None, 'B': None}
                for h, (X, tiles) in enumerate(seq):
                    if tiles:
                        last_idx[X] = h

                def emit_av(X, tiles, pt, is_last):
                    pyX, _, _, _, hX, _, _ = ctx[X]
                    for t, (kb, w, qo) in enumerate(tiles):
                        nc.tensor.matmul(
                            pyX[0:65, qo:qo + w],
                            v_sb[:, kb, hX, 0:65],
                            pt[:, t * QB + qo:t * QB + qo + w],
                            start=first[X] and t == 0,
                            stop=is_last and t == len(tiles) - 1)
                        if t == 0:
                            first[X] = False

                pend = {}  # h -> (X, tiles, pt)
                for h, (X, tiles) in enumerate(seq):
                    pyX, qX, kX, iX, hX, ytX, stag = ctx[X]
                    if tiles:
                        # segment t lives at cols [t*QB + qo, (t+1)*QB) of
                        # its chunk tile - bank-aligned, trimmed to the live
                        # causal range.  Dead columns are exp'd but never
                        # read by attn@v.
                        s = pp.tile([128, CH * QB], f32, tag=stag, bufs=1,
                                    name='s' + X)
                        for t, (kb, w, qo) in enumerate(tiles):
                            nc.tensor.matmul(
                                s[:, t * QB + qo:t * QB + qo + w],
                                kX[:, kb * KT:(kb + 1) * KT],
                                qX[:, iX * QB + qo:iX * QB + qo + w],
                                start=True, stop=True)
                    # attn@v of h-2 (same stream) keeps PE busy during exp(h)
                    if h - 2 in pend:
                        Xp, tp, ptp = pend.pop(h - 2)
                        emit_av(Xp, tp, ptp, last_idx[Xp] == h - 2)
                    if tiles:
                        wX = len(tiles) * QB
                        pt = wpool.tile([128, CH * QB], bf16, tag='pt', bufs=8,
                                        name='pt' + X)
                        nc.scalar.activation(pt[:, 0:wX], s[:, 0:wX], Exp)
                        for t, (kb, w, qo) in enumerate(tiles):
                            if kb - 4 * iX >= 0:
                                nc.vector.tensor_mul(
                                    pt[:, t * QB + qo:t * QB + qo + KT],
                                    pt[:, t * QB + qo:t * QB + qo + KT],
                                    msk_sb[:, :])
                        pend[h] = (X, tiles, pt)
                # drain
                for h in sorted(pend):
                    Xp, tp, ptp = pend[h]
                    emit_av(Xp, tp, ptp, last_idx[Xp] == h)
                normalize(pyA, ytA, slice(iA * QB, (iA + 1) * QB), 'sA')
                normalize(pyB, ytB, slice(iB * QB, (iB + 1) * QB), 'sB')

            # ---------- main loop ----------
            def outproj_block(i, ci=None, ib=None):
                for mm in range(QB // 128):
                    tok = i * QB + mm * 128
                    ot = wpool.tile([128, D], bf16, tag='ot', name='ot')
                    pos = [pp.tile([128, QB], f32, tag=('yA', 'yB')[dj],
                                   bufs=1, name='po') for dj in range(2)]
                    for dj, (d0, dw) in enumerate(((0, QB), (QB, 256))):
                        nc.tensor.matmul(pos[dj][:, 0:dw],
                                         yt01[:, tok:tok + 128],
                                         wo01_sb[:, d0:d0 + dw],
                                         start=True, stop=False)
                    for dj, (d0, dw) in enumerate(((0, QB), (QB, 256))):
                        nc.tensor.matmul(pos[dj][:, 0:dw],
                                         yt2[:, tok:tok + 128],
                                         wo2_sb[:, d0:d0 + dw],
                                         start=False, stop=True)
                    for dj, (d0, dw) in enumerate(((0, QB), (QB, 256))):
                        nc.vector.tensor_copy(ot[:, d0:d0 + dw],
                                              pos[dj][:, 0:dw])
                    # partial sums straight to DRAM; host reduces across the
                    # 4 cores of the batch group (gpsimd queue: keeps the
                    # sync DMA queue free for SBUF traffic)
                    nc.gpsimd.dma_start(out=out[tok:tok + 128, :],
                                        in_=ot[:, :])

            def reduce_scatter(ci):
                pass

            def attn_h01(i):
                attn_pair(qkA[0:64, 0:L], qkA[0:64, L:2 * L], i, yt[0], 0,
                          qkA[64:128, 0:L], qkA[64:128, L:2 * L], i, yt[1], 1)

            def attn_h2(p):
                attn_pair(qk2[0:64, 0:L], qk2[0:64, L:2 * L], 2 * p, yt[2], 2,
                          qk2[64:128, 0:L], qk2[64:128, L:2 * L], 2 * p + 1,
                          yt[2], 2)

            with tc.tile_pool(name='psum_proj', bufs=1, space='PSUM') as pjp:
                proj_head(pjp)

            with tc.tile_pool(name='psum', bufs=1, space='PSUM') as pp_:
                pp = pp_
                # phase 0 with the projection tail interleaved: the PE does
                # proj matmuls while ACT chews phase-0 exps
                attn_h01(0)
                proj_pair(2, 3)
                proj_v_range(8, 16)
                attn_h01(1)
                proj_pair(4, 5)
                proj_v_range(16, 24)
                attn_h2(0)
                proj_pair(6, 7)
                proj_v_range(24, 32)
                for p in (1, 2):
                    attn_h01(2 * p)
                    outproj_block(2 * (p - 1), p - 1, 2 * (p - 1))
                    attn_h01(2 * p + 1)
                    outproj_block(2 * p - 1, p - 1, 2 * (p - 1))
                    reduce_scatter(p - 1)
                    attn_h2(p)
                # phase 3 reordered: h2 before h01(7) so block 6 completes
                # early and its outproj+RS overlap the last attention pair
                attn_h01(6)
                outproj_block(4, 2, 4)
                attn_h2(3)
                outproj_block(5, 2, 4)
                reduce_scatter(2)
                outproj_block(6, 3, 6)
                reduce_scatter(3)
                attn_h01(7)
                outproj_block(7, 4, 7)
                reduce_scatter(4)
    nc.compile()
    return nc


def kernel(x, Wqkv, bqkv, Wo, bo):
    from concourse.bass_utils import run_bass_kernel_spmd

    if 'nc' not in _CACHE:
        _CACHE['nc'] = _build()
    nc = _CACHE['nc']

    bf = ml_dtypes.bfloat16
    x = np.asarray(x, np.float32)
    Wqkv = np.asarray(Wqkv, np.float32)
    bqkv = np.asarray(bqkv, np.float32)
    Wo = np.asarray(Wo, np.float32)
    bo = np.asarray(bo, np.float32)

    # device graph omits the qkv bias adds (always zeros per problem spec)
    assert np.abs(bqkv).max() == 0.0, "nonzero bqkv unsupported by this kernel"

    scale = 1.0 / np.sqrt(Dh)
    Q, K, V = Wqkv[:, 0:D], Wqkv[:, D:2 * D], Wqkv[:, 2 * D:3 * D]

    kl = np.arange(KT)[:, None]
    ql = np.arange(KT)[None, :]
    msk = np.ascontiguousarray(ql >= kl).astype(bf)

    in_maps = []
    for c in range(NCORES):
        b, g = divmod(c, 4)
        hs = [3 * g, 3 * g + 1, 3 * g + 2]
        cols = lambda W, h: W[:, h * Dh:(h + 1) * Dh]
        wqkv_np = np.concatenate(
            [cols(Q, hs[0]) * scale, cols(Q, hs[1]) * scale,
             cols(K, hs[0]), cols(K, hs[1]),
             cols(Q, hs[2]) * scale, cols(K, hs[2]),
             cols(V, hs[0]), cols(V, hs[1]), cols(V, hs[2])],
            axis=1).astype(bf)
        wo01_np = Wo[3 * g * Dh:(3 * g + 2) * Dh, :].astype(bf)
        wo2_np = Wo[(3 * g + 2) * Dh:(3 * g + 3) * Dh, :].astype(bf)
        xT_np = np.ascontiguousarray(x[b].T).astype(bf)
        in_maps.append({
            'xT': xT_np, 'wqkv': np.ascontiguousarray(wqkv_np),
            'wo01': np.ascontiguousarray(wo01_np),
            'wo2': np.ascontiguousarray(wo2_np),
            'msk': msk,
        })

    res = run_bass_kernel_spmd(nc, in_maps, core_ids=list(range(NCORES)))

    # each core returns the partial outproj sum for its 3 heads over the
    # full sequence; sum the 4 head-group cores of each batch
    out = np.empty((B, L, D), np.float32)
    for b in range(B):
        acc = res.results[4 * b]['out'].astype(np.float32)
        for g in range(1, 4):
            acc += res.results[4 * b + g]['out'].astype(np.float32)
        out[b] = acc
    out += bo[None, None, :]
    return out



# revision 12
# speedup vs baseline: 1.2507x; 1.1036x over previous
"""Causal self-attention (B=2, L=4096, D=768, H=12) on 8 TRN2 NeuronCores.

Sharding: core c -> batch b = c//4, head group g = c%4 (heads 3g..3g+2).
No collectives: each core writes its 3 heads' partial output projection for
the FULL sequence; the host sums the 4 head-group cores of each batch
(that is the unshard step) and adds bo.

Per-core structure (QB=256 q-blocks, 128-wide k-tiles, chunks of <=6):
- q/k stored per head as [128, 2L] with the lower/upper partition halves
  duplicated, so consecutive score matmuls (K=64) alternate PE row groups
  (0,0)/(64,0) and run CONCURRENTLY in the array (~2x score throughput).
  Chunk slots are permuted so each concurrent pair writes different PSUM
  banks.
- Two-stream staggered flash attention (A/B half-chunks) keeps the scalar
  engine's exp saturated: it is the critical path (~28M exp elements at
  1 elem/lane/cycle @1.2GHz ~= 190us minimum).
- PSUM: scores A (3 banks) + scores B (3) + shared pyA|pyB rowsum bank (1)
  + projection bank (1) = 8. QKV projection, v projection and the output
  projection are woven into the attention half-chunk slots as PE filler so
  the tensor engine never idles (keeps the HAM clock gate at 8/8).
- attn@v fuses the softmax denominator as a 65th ones-column of v; the
  reciprocal is broadcast across partitions with a K=1 matmul.
Host reassembles [2, 4096, 768] = sum of per-core partials + bo.
"""

import sys

for _p in ("/opt/trn_rl_repo",):
    if _p not in sys.path:
        sys.path.insert(0, _p)

import numpy as np
import ml_dtypes

B, L, D, H = 2, 4096, 768, 12
Dh = D // H          # 64
HPC = 3              # heads per core
NCORES = 8
QB = 256             # q block
KT = 128             # k tile
NQ = L // QB         # 16 q-blocks
KC = D // 128        # 6 contraction chunks for projections
CH = 6               # k-tiles per exp chunk (3 PSUM banks)

# chunk-slot permutations: concurrent score pairs (t, t+1) land in
# different PSUM banks (bank = slot//2); exp spans slots 0..max contiguously
SLOTS = {1: (0,), 2: (0, 2), 3: (0, 2, 1), 4: (0, 2, 1, 3),
         5: (0, 2, 1, 3, 4), 6: (0, 3, 1, 4, 2, 5)}

_CACHE = {}


def _tiles_for_block(b):
    """(kb, w, qo) per k-tile for q-block b (QB=256): the final k-tile is
    half-dead (only q cols 128:256 live), the one before needs the
    triangular mask."""
    out = []
    for kb in range(2 * b + 2):
        if kb == 2 * b + 1:
            out.append((kb, 128, 128))
        else:
            out.append((kb, 256, 0))
    return out


def _chunks_for_block(b):
    t = _tiles_for_block(b)
    return [t[i:i + CH] for i in range(0, len(t), CH)]


def _build():
    import concourse.mybir as mybir
    import concourse.tile as tile
    from concourse import bacc

    bf16 = mybir.dt.bfloat16
    f32 = mybir.dt.float32
    Exp = mybir.ActivationFunctionType.Exp

    nc = bacc.Bacc("TRN2", target_bir_lowering=False, debug=False,
                   num_devices=NCORES)

    xT = nc.dram_tensor('xT', [D, L], bf16, kind='ExternalInput')
    wqkv = nc.dram_tensor('wqkv', [D, 576], bf16, kind='ExternalInput')
    wo01 = nc.dram_tensor('wo01', [128, D], bf16, kind='ExternalInput')
    wo2 = nc.dram_tensor('wo2', [64, D], bf16, kind='ExternalInput')
    msk = nc.dram_tensor('msk', [KT, 256], bf16, kind='ExternalInput')
    out = nc.dram_tensor('out', [L, D], bf16, kind='ExternalOutput')

    with tile.TileContext(nc) as tc:
        with tc.tile_pool(name='const', bufs=1) as cpool, \
             tc.tile_pool(name='work', bufs=3) as wpool:

            # ---------------- load phase ----------------
            wq_sb = cpool.tile([128, KC, 576], bf16)
            for kc in range(KC):
                nc.sync.dma_start(out=wq_sb[:, kc, :],
                                  in_=wqkv[kc * 128:(kc + 1) * 128, :])
            tri = cpool.tile([KT, 256], bf16)
            nc.sync.dma_start(out=tri[:, :], in_=msk[:, :])
            # x in 512-col pieces: the first projection (tokens 0:512) can
            # start ~4us in instead of waiting for the full 6MB
            xt = cpool.tile([128, KC, L], bf16)
            for p0 in range(0, L, 512):
                for kc in range(KC):
                    nc.sync.dma_start(out=xt[:, kc, p0:p0 + 512],
                                      in_=xT[kc * 128:(kc + 1) * 128,
                                             p0:p0 + 512])
            wo01_sb = cpool.tile([128, D], bf16)
            nc.sync.dma_start(out=wo01_sb[:, :], in_=wo01[:, :])
            wo2_sb = cpool.tile([64, D], bf16)
            nc.sync.dma_start(out=wo2_sb[:, :], in_=wo2[:, :])
            ones = cpool.tile([128, 64], bf16)
            nc.vector.memset(ones[:, :], 1.0)

            # per-head q|k, duplicated across partition halves:
            # [0:64]  = q_h (cols 0:L) | k_h (cols L:2L)
            # [64:128] = same (feeds the (64,0) row-group of score pairs)
            qks = [cpool.tile([128, 2 * L], bf16, name=f'qk{h}')
                   for h in range(HPC)]
            v_sb = cpool.tile([128, L // KT, HPC, 65], bf16)
            nc.vector.memset(v_sb[:, :, :, 64:65], 1.0)
            yt01 = cpool.tile([128, L], bf16)
            yt2 = cpool.tile([64, L], bf16)
            yts = [yt01[0:64, :], yt01[64:128, :], yt2[0:64, :]]

            # ---------------- PSUM + proj/outproj steps ----------------
            pp = None    # set below (psum pool)
            pyAB = None  # [65, 512]: A rowsum block at cols 0:256, B at 256:512

            def qk_step(n, ct, tag='pj'):
                """q/k projection for token block n (512 wide), weight
                column chunk ct (0: q_h0|q_h1, 1: k_h0|k_h1, 2: q_h2|k_h2);
                result copied+duplicated into the qk tiles."""
                ps = pp.tile([128, 512], f32, tag=tag, bufs=1, name='pjqk')
                for kc in range(KC):
                    nc.tensor.matmul(ps[:, 0:512],
                                     wq_sb[:, kc, ct * 128:ct * 128 + 128],
                                     xt[:, kc, n * 512:(n + 1) * 512],
                                     start=(kc == 0), stop=(kc == KC - 1))
                st = wpool.tile([128, 512], bf16, tag='st', name='st')
                nc.vector.tensor_copy(st[:, :], ps[:, 0:512])
                qsl = slice(n * 512, (n + 1) * 512)
                ksl = slice(L + n * 512, L + (n + 1) * 512)
                if ct == 0:
                    dsts = [(qks[0], qsl, 0), (qks[1], qsl, 64)]
                elif ct == 1:
                    dsts = [(qks[0], ksl, 0), (qks[1], ksl, 64)]
                else:
                    dsts = [(qks[2], qsl, 0), (qks[2], ksl, 64)]
                for dst, sl, sp in dsts:
                    # same-partition half on DVE, cross-partition dup via DMA
                    # (vector queue: the sync queue is busy streaming x)
                    nc.vector.tensor_copy(dst[sp:sp + 64, sl], st[sp:sp + 64, :])
                    nc.scalar.dma_start(out=dst[64 - sp:128 - sp, sl],
                                        in_=st[sp:sp + 64, :])

            def v_step(m, tag='pj'):
                ps = pp.tile([128, 512], f32, tag=tag, bufs=1, name='pjv')
                for kc in range(KC):
                    nc.tensor.matmul(ps[:, 0:192],
                                     xt[:, kc, m * 128:(m + 1) * 128],
                                     wq_sb[:, kc, 384:576],
                                     start=(kc == 0), stop=(kc == KC - 1))
                nc.vector.tensor_copy(v_sb[:, m, :, 0:64], ps[:, 0:192])

            def outproj_wave(m, dj):
                d0, dw = ((0, 512), (512, 256))[dj]
                tok = m * 128
                ps = pp.tile([128, 512], f32, tag='pj', bufs=1, name='pjo')
                nc.tensor.matmul(ps[:, 0:dw], yt01[:, tok:tok + 128],
                                 wo01_sb[:, d0:d0 + dw], start=True, stop=False)
                nc.tensor.matmul(ps[:, 0:dw], yt2[:, tok:tok + 128],
                                 wo2_sb[:, d0:d0 + dw], start=False, stop=True)
                ot = wpool.tile([128, 512], bf16, tag='ot', name='ot')
                nc.vector.tensor_copy(ot[:, 0:dw], ps[:, 0:dw])
                nc.gpsimd.dma_start(out=out[tok:tok + 128, d0:d0 + dw],
                                    in_=ot[:, 0:dw])

            projq = []

            def emit_step(stp, tag='pj'):
                kind = stp[0]
                if kind == 'qk':
                    qk_step(stp[1], stp[2], tag)
                elif kind == 'v':
                    v_step(stp[1], tag)
                else:
                    outproj_wave(stp[1], stp[2])

            def filler():
                k = 2 if len(projq) > 10 else 1
                for _ in range(min(k, len(projq))):
                    emit_step(projq.pop(0))

            def flush_proj():
                # everything except outproj waves must land before the next
                # group reads it
                i = 0
                while i < len(projq):
                    if projq[i][0] in ('qk', 'v'):
                        emit_step(projq.pop(i))
                    else:
                        i += 1

            # ---------------- attention ----------------
            def normalize(X, ctx):
                off, hX, bX, ytX, done = ctx[X]
                rs = wpool.tile([1, 256], bf16, tag='rs', name='rs')
                nc.vector.tensor_copy(rs[:, :], pyAB[64:65, off:off + 256])
                pb = pp.tile([128, 512], f32, tag='pj', bufs=1, name='pb')
                nc.tensor.matmul(pb[0:64, 0:256], ones[0:1, 0:64],
                                 rs[0:1, :], start=True, stop=True)
                rcp = wpool.tile([64, 256], f32, tag='rcp', name='rcp')
                nc.vector.reciprocal_approx_fast(out=rcp[:, :],
                                                 in_=pb[0:64, 0:256])
                nc.vector.tensor_mul(ytX[:, bX * QB:(bX + 1) * QB],
                                     pyAB[0:64, off:off + 256],
                                     rcp[:, :])
                if done is not None:
                    done()

            def attn_pair(hA, bA, hB, bB, doneA=None, doneB=None):
                """Two causal streams, staggered half-chunk pipeline.
                Scores are emitted as row-group-alternating pairs so the PE
                computes two K=64 tiles concurrently."""
                chA = _chunks_for_block(bA)
                chB = _chunks_for_block(bB)
                nch = max(len(chA), len(chB))
                chA = chA + [[]] * (nch - len(chA))
                chB = chB + [[]] * (nch - len(chB))
                seq = []
                for j in range(nch):
                    seq.append(('A', chA[j]))
                    seq.append(('B', chB[j]))
                ctx = {'A': (0, hA, bA, yts[hA], doneA),
                       'B': (256, hB, bB, yts[hB], doneB)}
                # start=True clears the WHOLE psum bank (not just the
                # addressed elements) -> only the very first av of the pair
                # may carry it; the other stream's first write still lands
                # fresh because the clear reset its has_written bits
                first = {'pair': True}
                last_idx = {}
                for h, (X, tiles) in enumerate(seq):
                    if tiles:
                        last_idx[X] = h

                def emit_av(X, tiles, pt, is_last):
                    off, hX, bX, ytX, _ = ctx[X]
                    n = len(tiles)
                    smap = SLOTS[n]
                    for t, (kb, w, qo) in enumerate(tiles):
                        c0 = smap[t] * 256 + qo
                        nc.tensor.matmul(
                            pyAB[0:65, off + qo:off + qo + w],
                            v_sb[:, kb, hX, 0:65], pt[:, c0:c0 + w],
                            start=first['pair'] and t == 0,
                            stop=is_last and t == n - 1,
                            skip_group_check=True)
                        if t == 0:
                            first['pair'] = False
                    if is_last:
                        normalize(X, ctx)

                pend = {}
                for h, (X, tiles) in enumerate(seq):
                    off, hX, bX, ytX, _ = ctx[X]
                    if tiles:
                        n = len(tiles)
                        smap = SLOTS[n]
                        s = pp.tile([128, CH * 256], f32, tag='s' + X,
                                    bufs=1, name='s' + X)
                        qk = qks[hX]
                        for t, (kb, w, qo) in enumerate(tiles):
                            hp = (t % 2) * 64
                            c0 = smap[t] * 256 + qo
                            nc.tensor.matmul(
                                s[:, c0:c0 + w],
                                qk[hp:hp + 64, L + kb * KT:L + (kb + 1) * KT],
                                qk[hp:hp + 64, bX * QB + qo:bX * QB + qo + w],
                                start=True, stop=True)
                    if h - 2 in pend:
                        Xp, tp, ptp = pend.pop(h - 2)
                        emit_av(Xp, tp, ptp, last_idx[Xp] == h - 2)
                    if tiles:
                        span = (max(smap[:n]) + 1) * 256
                        pt = wpool.tile([128, CH * 256], bf16, tag='pt',
                                        bufs=6, name='pt' + X)
                        nc.scalar.activation(pt[:, 0:span], s[:, 0:span], Exp)
                        for t, (kb, w, qo) in enumerate(tiles):
                            c0 = smap[t] * 256
                            if kb == 2 * bX:
                                nc.vector.tensor_mul(pt[:, c0:c0 + 256],
                                                     pt[:, c0:c0 + 256],
                                                     tri[:, 0:256])
                            elif kb == 2 * bX + 1:
                                nc.vector.tensor_mul(pt[:, c0 + 128:c0 + 256],
                                                     pt[:, c0 + 128:c0 + 256],
                                                     tri[:, 0:128])
                        pend[h] = (X, tiles, pt)
                    filler()
                for h in sorted(pend):
                    Xp, tp, ptp = pend[h]
                    emit_av(Xp, tp, ptp, last_idx[Xp] == h)

            # ---------- main loop ----------
            with tc.tile_pool(name='psum', bufs=1, space='PSUM') as pp_:
                pp = pp_
                pyAB = pp.tile([65, 512], f32, tag='py', bufs=1, name='pyAB')
                # lead-in: everything attention group 0 needs, rotating
                # through the (still free) attention psum banks
                lead = [('qk', 0, 0), ('qk', 0, 1), ('v', 0), ('v', 1),
                        ('qk', 0, 2), ('v', 2), ('v', 3)]
                for i, stp in enumerate(lead):
                    emit_step(stp, tag=('sA', 'sB', 'pj')[i % 3])

                for p in range(8):
                    if p < 7:
                        n = p + 1
                        projq.extend([('qk', n, 0), ('qk', n, 1),
                                      ('qk', n, 2), ('v', 4 * n), ('v', 4 * n + 1),
                                      ('v', 4 * n + 2), ('v', 4 * n + 3)])
                    attn_pair(0, 2 * p, 1, 2 * p)
                    attn_pair(0, 2 * p + 1, 1, 2 * p + 1)

                    def doneA(p=p):
                        # block 2p fully normalized for all 3 heads ->
                        # its two outproj tiles can go now (overlaps the
                        # h2 B-stream tail)
                        for m in (4 * p, 4 * p + 1):
                            for dj in (0, 1):
                                outproj_wave(m, dj)

                    def doneB(p=p):
                        for m in (4 * p + 2, 4 * p + 3):
                            for dj in (0, 1):
                                projq.append(('op', m, dj))

                    attn_pair(2, 2 * p, 2, 2 * p + 1, doneA=doneA, doneB=doneB)
                    flush_proj()
                while projq:
                    emit_step(projq.pop(0))
    nc.compile()
    return nc


def kernel(x, Wqkv, bqkv, Wo, bo):
    from concourse.bass_utils import run_bass_kernel_spmd

    if 'nc' not in _CACHE:
        _CACHE['nc'] = _build()
    nc = _CACHE['nc']

    bf = ml_dtypes.bfloat16
    x = np.asarray(x, np.float32)
    Wqkv = np.asarray(Wqkv, np.float32)
    bqkv = np.asarray(bqkv, np.float32)
    Wo = np.asarray(Wo, np.float32)
    bo = np.asarray(bo, np.float32)

    # device graph omits the qkv bias adds (always zeros per problem spec)
    assert np.abs(bqkv).max() == 0.0, "nonzero bqkv unsupported by this kernel"

    scale = 1.0 / np.sqrt(Dh)
    Q, K, V = Wqkv[:, 0:D], Wqkv[:, D:2 * D], Wqkv[:, 2 * D:3 * D]

    # triangular mask [128, 256]: col j live for partition r when j >= r
    msk = np.ascontiguousarray(
        np.arange(256)[None, :] >= np.arange(KT)[:, None]).astype(bf)

    in_maps = []
    for c in range(NCORES):
        b, g = divmod(c, 4)
        hs = [3 * g, 3 * g + 1, 3 * g + 2]
        cols = lambda W, h: W[:, h * Dh:(h + 1) * Dh]
        wqkv_np = np.concatenate(
            [cols(Q, hs[0]) * scale, cols(Q, hs[1]) * scale,
             cols(K, hs[0]), cols(K, hs[1]),
             cols(Q, hs[2]) * scale, cols(K, hs[2]),
             cols(V, hs[0]), cols(V, hs[1]), cols(V, hs[2])],
            axis=1).astype(bf)
        wo01_np = Wo[3 * g * Dh:(3 * g + 2) * Dh, :].astype(bf)
        wo2_np = Wo[(3 * g + 2) * Dh:(3 * g + 3) * Dh, :].astype(bf)
        xT_np = np.ascontiguousarray(x[b].T).astype(bf)
        in_maps.append({
            'xT': xT_np, 'wqkv': np.ascontiguousarray(wqkv_np),
            'wo01': np.ascontiguousarray(wo01_np),
            'wo2': np.ascontiguousarray(wo2_np),
            'msk': msk,
        })

    res = run_bass_kernel_spmd(nc, in_maps, core_ids=list(range(NCORES)))

    # each core returns the partial outproj sum for its 3 heads over the
    # full sequence; sum the 4 head-group cores of each batch
    out = np.empty((B, L, D), np.float32)
    for b in range(B):
        acc = res.results[4 * b]['out'].astype(np.float32)
        for g in range(1, 4):
            acc += res.results[4 * b + g]['out'].astype(np.float32)
        out[b] = acc
    out += bo[None, None, :]
    return out
